# revision 27
# baseline (speedup 1.0000x reference)
"""GNN message passing (GCNConv -> global mean pool -> dense softmax) on 8 TRN2 cores.

Sharding: graphs are partitioned by seg_ids so each core owns 8 whole graphs
(a contiguous node range).  Edges are routed to the core that owns their
destination node; each core gathers source-node features (bf16 x rows, 256B)
straight from HBM with dma_gather, aggregates messages into per-window PSUM
tiles via one-hot matmuls on the TensorEngine, then projects with W1, applies
bias+relu, segment-mean pools and runs the dense softmax head locally.  No
collectives.

v2 vs v1: everything bf16 (gather rows 256B instead of 512B, matmuls at full
PE rate), 64-slot destination windows (halves the DVE one-hot build cost at
the same edge padding), one-hot selectors built in two batched DVE ops per
gather call (4D access patterns with stride-1 size-2 innermost dims so the
DVE 2x mode engages; dsub/wv shipped duplicated in pairs; eq built in-place
in the oh tile), window-outer / group-inner accumulation so each window's
PSUM is evacuated once (on the otherwise idle ACT engine), gather calls of
8 windows x 4 chunks with a 10-deep gather-buffer pool to keep the DMA
descriptor stream saturated, and per-block src-sorted gather indices for
ascending HBM addresses.

Measured (KREP-slope): main loop ~1.26-1.31 ms/exec vs 3.17 ms baseline;
the dma_gather descriptor stream (one 256B descriptor per edge, ~2 ns/desc
+ ~200 GB/s) is the hard floor; all compute hides under it.
"""

import os
import sys

sys.path.insert(0, "/opt/trn_rl_repo")

import numpy as np
import ml_dtypes

import concourse.bass as bass
import concourse.bacc as bacc
import concourse.mybir as mybir
import concourse.tile as tile
from concourse.bass import broadcast_tensor_aps
from concourse.bass_utils import run_bass_kernel_spmd

N_CORES = 8
N_GRAPHS = 64
G_PER_CORE = N_GRAPHS // N_CORES
P = 128                                     # edges per chunk (PE contraction)
SW = int(os.environ.get("KSW", "64"))       # dst slots per window
BW = int(os.environ.get("KBW", "8"))        # windows per gather-call block
N_QUEUES = int(os.environ.get("KQ", "4"))
KGB = int(os.environ.get("KGB", "10"))      # gather buffer count
KREP = int(os.environ.get("KREP", "1"))     # main-loop repeats (timing only)
KSP = bool(int(os.environ.get("KSP", "0")))  # dma_gather single_packet
KGDT = os.environ.get("KGDT", "bf16")       # x dtype: bf16 | f32 (abl only)
KHALF = bool(int(os.environ.get("KHALF", "0")))  # gather half descs (abl)
MAX_I16 = 32768
BF16 = ml_dtypes.bfloat16


def _balance_windows(deg4, W, cap):
    """Assign each dst (rows of deg4 [n,4]) to one of W windows, <=cap dsts
    per window, minimizing the max per-(group,window) edge load."""
    n = deg4.shape[0]
    order = np.argsort(-deg4.sum(1), kind="stable")
    load = np.zeros((W, 4), np.int64)
    slots = np.zeros(W, np.int64)
    win = np.zeros(n, np.int64)
    for d in order:
        free = slots < cap
        cand = (load[free] + deg4[d]).max(1)
        wsel = np.flatnonzero(free)[np.argmin(cand)]
        win[d] = wsel
        load[wsel] += deg4[d]
        slots[wsel] += 1
    return win, load


def _prep_core(es, dl, ew, n_loc, W, gbase):
    g = (es // gbase).astype(np.int64)
    deg4 = np.zeros((max(n_loc, 1), 4), np.int64)
    np.add.at(deg4, (dl, g), 1)
    win, load = _balance_windows(deg4, W, SW)
    slot_of = np.zeros(max(n_loc, 1), np.int64)
    for w in range(W):
        members = np.flatnonzero(win == w)
        slot_of[members] = np.arange(len(members))
    row_of = win * SW + slot_of
    q_req = int(np.ceil(load.max() / P)) if load.size else 1
    return {"g": g, "win": win[dl], "slot": slot_of[dl], "row_of": row_of,
            "q_req": max(q_req, 1)}


def _build_schedule(core, q, W, calls):
    """Fill offs/dsub/wv [P, C] for one core.  Column order: call-by-call,
    where call (b, g) covers (w, j) for w in block b, j in 0..q-1."""
    g, win, slot = core["g"], core["win"], core["slot"]
    C = 4 * W * q
    offs = np.zeros((P, C), np.int16)
    dsub = np.full((P, C), -1.0, np.float32)
    wv = np.zeros((P, C), np.float32)
    # per-edge chunk position: order edges of each (g, w) block by src id so
    # each gather call walks ascending HBM addresses; edge m of the block
    # goes to chunk j=m//P, row p=m%P
    key = (g * W + win).astype(np.int64) * (2 ** 32) + core["es_sorted"]
    order = np.argsort(key, kind="stable")
    gs, ws = g[order], win[order]
    blk = gs * W + ws
    changes = np.r_[True, blk[1:] != blk[:-1]]
    block_start = np.maximum.accumulate(np.where(changes, np.arange(len(blk)), 0))
    pos = np.arange(len(blk)) - block_start
    if np.any(pos // P >= q):
        raise RuntimeError("window overflow: q too small")
    # column of chunk (g, w, j) under the call layout
    colof = np.zeros((4, W, q), np.int64)
    c0 = 0
    for (b, gg, wlist) in calls:
        for w in wlist:
            for j in range(q):
                colof[gg, w, j] = c0
                c0 += 1
    assert c0 == C
    k = colof[gs, ws, pos // P]
    p = pos % P
    offs[p, k] = (core["es_sorted"][order] - gs * core["gbase"]).astype(np.int16)
    dsub[p, k] = slot[order].astype(np.float32)
    wv[p, k] = core["ew_sorted"][order]
    return offs, dsub, wv


def _wrap_idx(offs, calls, q):
    """Pack per-call int16 index lists in the HW wrap-16 layout, 8x
    replicated.  Returns [128, total/16]."""
    cols = []
    c0 = 0
    for (b, g, wlist) in calls:
        nch = len(wlist) * q
        ii = offs[:, c0:c0 + nch].T.reshape(-1)   # i = chunk*128 + p
        cols.append(ii.reshape(-1, 16).T)         # [16, n/16]
        c0 += nch
    arr = np.concatenate(cols, axis=1)
    return np.tile(arr, (8, 1)).astype(np.int16)


def _make_calls(W):
    """Call list: block b of BW windows x 4 groups."""
    calls = []
    nb = (W + BW - 1) // BW
    for b in range(nb):
        wlist = list(range(b * BW, min((b + 1) * BW, W)))
        for g in range(4):
            calls.append((b, g, wlist))
    return calls


def _prepare(x, edge_src, edge_dst, edge_weight, seg_ids, W1, b1, W2, b2):
    N = x.shape[0]
    gbase = int(np.ceil(N / 4))
    assert gbase <= MAX_I16
    bounds = np.searchsorted(seg_ids, np.arange(0, N_GRAPHS + 1, G_PER_CORE))
    n_locs = np.diff(bounds)
    # pad the window count: with zero slack the last dsts get forced into
    # whatever window has free slots, blowing its max group load (and q)
    W = int(np.ceil(n_locs.max() / SW)) + int(os.environ.get("KWP", "4"))
    core_of_edge = np.searchsorted(bounds, edge_dst, side="right") - 1
    calls = _make_calls(W)

    cores = []
    for c in range(N_CORES):
        m = core_of_edge == c
        es, ed, ew = edge_src[m], edge_dst[m] - bounds[c], edge_weight[m]
        info = _prep_core(es, ed, ew, int(n_locs[c]), W, gbase)
        info.update(es_sorted=es, ew_sorted=ew, gbase=gbase)
        cores.append(info)
    q = max(ci["q_req"] for ci in cores)
    C = 4 * W * q

    x_bf = np.ascontiguousarray(x.astype(BF16 if KGDT == "bf16" else np.float32))
    iota = np.tile(np.arange(SW, dtype=np.float32), (P, 1)).astype(BF16)
    ident = np.eye(P, dtype=np.float32).astype(BF16)
    b1b = np.tile(b1[None, :], (SW, 1)).astype(BF16)
    b2b = np.tile(b2[None, :], (G_PER_CORE, 1)).astype(np.float32)
    w1_bf = W1.astype(BF16)
    w2_bf = W2.astype(BF16)

    in_maps = []
    for c in range(N_CORES):
        ci = cores[c]
        offs, dsub, wv = _build_schedule(ci, q, W, calls)
        idx16 = _wrap_idx(offs, calls, q)
        # pooling matrix [SW, W, G_PER_CORE]
        pool = np.zeros((SW, W, G_PER_CORE), np.float32)
        segs_loc = seg_ids[bounds[c]:bounds[c + 1]] - c * G_PER_CORE
        rows = ci["row_of"][:n_locs[c]]
        pool[rows % SW, rows // SW, segs_loc] = 1.0
        cnts = np.bincount(segs_loc, minlength=G_PER_CORE).astype(np.float32)
        invc = np.tile((1.0 / np.maximum(cnts, 1.0))[None, :], (64, 1)).astype(np.float32)
        dsub2 = np.repeat(dsub.astype(BF16)[:, :, None], 2, axis=2)
        wv2 = np.repeat(wv.astype(BF16)[:, :, None], 2, axis=2)
        in_maps.append({
            "x": x_bf,
            "idx16": idx16,
            "dsub": dsub2.reshape(P, 2 * C), "wv": wv2.reshape(P, 2 * C),
            "pool": pool.reshape(SW, W * G_PER_CORE).astype(BF16),
            "invc": invc,
            "W1": w1_bf, "b1b": b1b, "W2": w2_bf, "b2b": b2b,
            "iota": iota, "ident": ident,
        })
    meta = {"N": N, "W": W, "q": q, "C": C, "gbase": gbase,
            "idx_cols": in_maps[0]["idx16"].shape[1]}
    return in_maps, meta


def _build_program(meta):
    N, W, q, gbase = meta["N"], meta["W"], meta["q"], meta["gbase"]
    C = meta["C"]
    f32 = mybir.dt.float32
    bf16 = mybir.dt.bfloat16
    nc = bacc.Bacc("TRN2", target_bir_lowering=False, debug=False,
                   num_devices=N_CORES, num_swdge_queues=N_QUEUES)
    gdt = bf16 if KGDT == "bf16" else f32
    x = nc.declare_dram_parameter("x", [N, 128], gdt, isOutput=False)
    idx16 = nc.declare_dram_parameter("idx16", [128, meta["idx_cols"]],
                                      mybir.dt.int16, isOutput=False)
    dsub = nc.declare_dram_parameter("dsub", [128, 2 * C], bf16, isOutput=False)
    wv = nc.declare_dram_parameter("wv", [128, 2 * C], bf16, isOutput=False)
    pool_p = nc.declare_dram_parameter("pool", [SW, W * G_PER_CORE], bf16,
                                       isOutput=False)
    invc_p = nc.declare_dram_parameter("invc", [64, G_PER_CORE], f32, isOutput=False)
    W1_p = nc.declare_dram_parameter("W1", [128, 64], bf16, isOutput=False)
    b1b_p = nc.declare_dram_parameter("b1b", [SW, 64], bf16, isOutput=False)
    W2_p = nc.declare_dram_parameter("W2", [64, 4], bf16, isOutput=False)
    b2b_p = nc.declare_dram_parameter("b2b", [G_PER_CORE, 4], f32, isOutput=False)
    iota_p = nc.declare_dram_parameter("iota", [128, SW], bf16, isOutput=False)
    ident_p = nc.declare_dram_parameter("ident", [128, 128], bf16, isOutput=False)
    probs = nc.declare_dram_parameter("probs", [G_PER_CORE, 4], f32, isOutput=True)

    calls = _make_calls(W)
    eq_op = mybir.AluOpType.is_equal
    mul_op = mybir.AluOpType.mult

    with tile.TileContext(nc) as tc:
        with tc.tile_pool(name="const", bufs=1) as cp, \
             tc.tile_pool(name="agg", bufs=1) as aggp, \
             tc.tile_pool(name="stream", bufs=1) as sp:
            iota_t = cp.tile([128, SW], bf16)
            nc.sync.dma_start(iota_t[:], iota_p[:])
            ident_t = cp.tile([128, 128], bf16)
            nc.sync.dma_start(ident_t[:], ident_p[:])
            w1_t = cp.tile([128, 64], bf16)
            nc.sync.dma_start(w1_t[:], W1_p[:])
            b1b_t = cp.tile([SW, 64], bf16)
            nc.sync.dma_start(b1b_t[:], b1b_p[:])
            w2_t = cp.tile([64, 4], bf16)
            nc.sync.dma_start(w2_t[:], W2_p[:])
            b2b_t = cp.tile([G_PER_CORE, 4], f32)
            nc.sync.dma_start(b2b_t[:], b2b_p[:])
            invc_t = cp.tile([64, G_PER_CORE], f32)
            nc.sync.dma_start(invc_t[:], invc_p[:])
            pool_t = cp.tile([SW, W, G_PER_CORE], bf16)
            nc.sync.dma_start(pool_t[:], pool_p[:].rearrange(
                "p (w f) -> p w f", f=G_PER_CORE))
            dsub_t = sp.tile([128, C, 2], bf16)
            nc.sync.dma_start(dsub_t[:], dsub[:].rearrange("p (c two) -> p c two", two=2))
            wv_t = sp.tile([128, C, 2], bf16)
            nc.sync.dma_start(wv_t[:], wv[:].rearrange("p (c two) -> p c two", two=2))
            agg_t = aggp.tile([SW, W, 128], bf16)
            _kabl = os.environ.get("KABL", "full")
            if _kabl in ("gather", "noga"):
                nc.vector.memset(agg_t[:], 0.0)
            oh_const = None
            if _kabl == "nooh":
                oh_const = cp.tile([128, BW * q, SW], bf16)
                nc.vector.memset(oh_const[:], 0.0)

            # ---- main loop: gather + one-hot matmuls into window PSUMs ----
            with tc.tile_pool(name="gbuf", bufs=KGB) as gp, \
                 tc.tile_pool(name="idxs", bufs=4) as ixp, \
                 tc.tile_pool(name="oh", bufs=int(os.environ.get("KOB", "5"))) as ohp, \
                 tc.tile_pool(name="wpsum", bufs=6, space="PSUM") as wpp:
                for rep in range(KREP):
                    call_i = 0
                    col0 = 0
                    idx_col0 = 0
                    nb = (W + BW - 1) // BW
                    for b in range(nb):
                        wlist = list(range(b * BW, min((b + 1) * BW, W)))
                        nch = len(wlist) * q
                        nidx = nch * 128
                        gbufs = []
                        ohs = []
                        bcol0 = col0
                        KABL = os.environ.get("KABL", "full")
                        for g in range(4):
                            xg = x[g * gbase:min((g + 1) * gbase, N), :]
                            ixt = ixp.tile([128, BW * q * 8], mybir.dt.int16,
                                           tag="ix")
                            nc.sync.dma_start(
                                ixt[:, :nidx // 16],
                                idx16[:, idx_col0:idx_col0 + nidx // 16])
                            gbuf = gp.tile([128, BW * q, 128], gdt, tag="g")
                            if KABL != "noga":
                                gn = nidx // 2 if KHALF else nidx
                                nc.gpsimd.dma_gather(
                                    gbuf[:, :gn // 128, :], xg,
                                    ixt[:, :gn // 16],
                                    gn, gn, 128,
                                    single_packet=KSP,
                                    queue_num=call_i % N_QUEUES)
                            oh_t = (oh_const if KABL == "nooh" else
                                    ohp.tile([128, BW * q, SW], bf16, tag="oh"))
                            if KABL not in ("gather", "nooh"):
                                # batched one-hot build; 4D APs keep every
                                # operand's innermost dim stride-1 size-2 so
                                # the DVE 2x mode engages (dsub/wv are shipped
                                # duplicated in pairs for this).  eq is built
                                # in-place in oh_t, then scaled by wv.
                                i_ap = iota_t[:]
                                i_b = bass.AP(i_ap.tensor, i_ap.offset, [
                                    i_ap.ap[0], [0, nch], [2, SW // 2], [1, 2]])
                                d_ap = dsub_t[:, col0:col0 + nch, :]
                                d_b = bass.AP(d_ap.tensor, d_ap.offset, [
                                    d_ap.ap[0], [2, nch], [0, SW // 2], [1, 2]])
                                w_ap = wv_t[:, col0:col0 + nch, :]
                                w_b = bass.AP(w_ap.tensor, w_ap.offset, [
                                    w_ap.ap[0], [2, nch], [0, SW // 2], [1, 2]])
                                o_ap = oh_t[:, :nch, :]
                                o4 = bass.AP(o_ap.tensor, o_ap.offset, [
                                    o_ap.ap[0], [SW, nch], [2, SW // 2], [1, 2]])
                                nc.vector.tensor_tensor(o4, i_b, d_b, eq_op)
                                nc.vector.tensor_tensor(o4, o4, w_b, mul_op)
                            gbufs.append(gbuf)
                            ohs.append(oh_t)
                            idx_col0 += nidx // 16
                            col0 += nch
                            call_i += 1
                        if KABL in ("gather", "noga"):
                            continue
                        for wi, w in enumerate(wlist):
                            psum = wpp.tile([SW, 128], f32)
                            for g in range(4):
                                for j in range(q):
                                    m = wi * q + j
                                    nc.tensor.matmul(
                                        psum[:], ohs[g][:, m, :],
                                        gbufs[g][:, m, :],
                                        start=(g == 0 and j == 0),
                                        stop=(g == 3 and j == q - 1))
                            nc.scalar.copy(agg_t[:, w, :], psum[:])

            # ---- post: h = relu(agg @ W1 + b1); pool; head; softmax ----
            with tc.tile_pool(name="post", bufs=3) as pp, \
                 tc.tile_pool(name="tpsum", bufs=2, space="PSUM") as tpp, \
                 tc.tile_pool(name="hpsum", bufs=2, space="PSUM") as hpp, \
                 tc.tile_pool(name="ppsum", bufs=1, space="PSUM") as ppp:
                pool_psum = ppp.tile([64, G_PER_CORE], f32)
                for w in range(W):
                    tp = tpp.tile([128, SW], bf16)
                    nc.tensor.transpose(tp[:], agg_t[:, w, :],
                                        ident_t[:SW, :SW])
                    aT = pp.tile([128, SW], bf16, tag="aT")
                    nc.scalar.copy(aT[:], tp[:])
                    hp = hpp.tile([SW, 64], f32)
                    nc.tensor.matmul(hp[:], aT[:], w1_t[:], start=True, stop=True)
                    h = pp.tile([SW, 64], bf16, tag="h")
                    nc.vector.tensor_add(h[:], hp[:], b1b_t[:])
                    nc.vector.tensor_scalar_max(h[:], h[:], 0.0)
                    nc.tensor.matmul(pool_psum[:], h[:], pool_t[:, w, :],
                                     start=(w == 0), stop=(w == W - 1))
                pooled = pp.tile([64, G_PER_CORE], bf16, tag="pl")
                nc.vector.tensor_mul(pooled[:], pool_psum[:], invc_t[:])
                lg_psum = ppp.tile([G_PER_CORE, 4], f32)
                nc.tensor.matmul(lg_psum[:], pooled[:], w2_t[:], start=True,
                                 stop=True)
                lg = pp.tile([G_PER_CORE, 4], f32, tag="lg")
                nc.vector.tensor_add(lg[:], lg_psum[:], b2b_t[:])
                mx = pp.tile([G_PER_CORE, 1], f32, tag="mx")
                nc.vector.reduce_max(mx[:], lg[:], axis=mybir.AxisListType.X)
                nc.vector.tensor_scalar(lg[:], lg[:], mx[:], None,
                                        mybir.AluOpType.subtract)
                ex = pp.tile([G_PER_CORE, 4], f32, tag="ex")
                nc.scalar.activation(ex[:], lg[:], mybir.ActivationFunctionType.Exp)
                sm = pp.tile([G_PER_CORE, 1], f32, tag="sm")
                nc.vector.reduce_sum(sm[:], ex[:], axis=mybir.AxisListType.X)
                rc = pp.tile([G_PER_CORE, 1], f32, tag="rc")
                nc.vector.reciprocal(rc[:], sm[:])
                ot = pp.tile([G_PER_CORE, 4], f32, tag="ot")
                nc.vector.tensor_scalar(ot[:], ex[:], rc[:], None,
                                        mybir.AluOpType.mult)
                nc.sync.dma_start(probs[:], ot[:])
    nc.compile()
    return nc


def kernel(x, edge_src, edge_dst, edge_weight, seg_ids, W1, b1, W2, b2):
    x = np.asarray(x, np.float32)
    in_maps, meta = _prepare(
        x, np.asarray(edge_src), np.asarray(edge_dst),
        np.asarray(edge_weight, np.float32), np.asarray(seg_ids),
        np.asarray(W1, np.float32), np.asarray(b1, np.float32),
        np.asarray(W2, np.float32), np.asarray(b2, np.float32))
    nc = _build_program(meta)
    res = run_bass_kernel_spmd(nc, in_maps, core_ids=list(range(N_CORES)))
    return np.concatenate([res.results[c]["probs"] for c in range(N_CORES)], axis=0)


if __name__ == "__main__":
    pass
